# revision 37
# baseline (speedup 1.0000x reference)
"""GroupedQueryAttention Trainium2 kernel (8 NeuronCores, SPMD).

Sharding: 16 (batch, q-head) pairs over 8 cores -> core c handles batch c//4,
kv-head j=c%4, q-heads {2j, 2j+1}. Each core computes its heads' causal flash
attention plus its partial output projection; host sums the 4 partials per
batch.

Device-side layout is fully "transposed" (head_dim on partitions, sequence on
free dim) so no on-chip transposes are needed anywhere:
  scores^T[k, q] = kT_chunk.T @ qT   (row-tiled matmuls, K=32)
  P^T = exp(scores^T * 1/sqrt(hd))   (single fused ACT op per 512-k group)
  out^T[hd, q]  = v_aug.T @ P^T      (v_aug has a leading ones column ->
                                      row 0 of the accumulator is the softmax
                                      denominator, for free)
RoPE is applied with zero shuffles by projecting hidden states twice: once
with W and once with (perm+sign) W, then q' = qT*cos + qrotT*sin.

v2: all matmul operands in bf16 (fp32 matmuls cost 4 cycles/row on the PE,
bf16 cost 1 -> was tensor-bound), PV matmuls issued one pipeline step behind
the exp so the PE never stalls on the scalar engine, next block's projections
prefetched during the current block's attention, and the causal triangle
masks offloaded to gpsimd (DVE was 17% busy).
"""

import json
import sys

import numpy as np
import ml_dtypes

for _p in ("/opt/trn_rl_repo",):
    if _p not in sys.path:
        try:
            import concourse.bass  # noqa: F401
        except Exception:
            sys.path.insert(0, _p)
    break

import concourse.bass as bass
import concourse.tile as tile
from concourse import mybir
from concourse.bass_utils import run_bass_kernel_spmd

F32 = mybir.dt.float32
F32R = mybir.dt.float32r
BF16 = mybir.dt.bfloat16
NP_BF16 = ml_dtypes.bfloat16

B, S, H = 2, 4096, 256
NH, NKV, HD = 8, 4, 32
QB = 512                   # q block width
NQB = S // QB              # 8
KC = 128                   # k chunk
SCALE = 1.0 / np.sqrt(HD)
ROPE_BASE = 10000.0


# ---------------------------------------------------------------- wait fixup
def _fix_waits_json(bir_bytes: bytes) -> bytes:
    """walrus (gen3) allows only one sync-wait per instruction struct; hoist
    extra waits onto inserted same-engine NoOps."""
    m = json.loads(bir_bytes)
    counter = 0
    for f in m.get("functions", []):
        for blk in f.get("blocks", []):
            out = []
            for inst in blk.get("instructions", []):
                si = inst.get("sync_info") or {}
                waits = si.get("on_wait") or []
                keep = 0 if inst.get("opcode") == "Matmult" else 1
                if len(waits) > keep:
                    for wsub in waits[keep:]:
                        counter += 1
                        out.append({
                            "debug": inst.get("debug", 0),
                            "engine": inst["engine"],
                            "ins": [],
                            "outs": [],
                            "name": f"waitfix-{counter}",
                            "opcode": "NoOp",
                            "sync_info": {"on_update": [], "on_wait": [wsub]},
                        })
                    si["on_wait"] = waits[:keep]
                out.append(inst)
            blk["instructions"] = out
    return json.dumps(m).encode()


def _install_waitfix(nc):
    orig = nc.to_json_bytes

    def patched(*a, **k):
        return _fix_waits_json(orig(*a, **k))

    nc.to_json_bytes = patched


# ---------------------------------------------------------------- device code
def _build_module():
    nc = bass.Bass()

    hsT = nc.declare_dram_parameter("hsT", [H, S], BF16, isOutput=False)
    wqkT = nc.declare_dram_parameter("wqkT", [H, 96], BF16, isOutput=False)
    wqkrotT = nc.declare_dram_parameter("wqkrotT", [H, 96], BF16, isOutput=False)
    wvT = nc.declare_dram_parameter("wvT", [H, HD], BF16, isOutput=False)
    gt0 = nc.declare_dram_parameter("gt0", [HD + 1, H], BF16, isOutput=False)
    gt1 = nc.declare_dram_parameter("gt1", [HD + 1, H], BF16, isOutput=False)
    cosT = nc.declare_dram_parameter("cosT", [96, S], BF16, isOutput=False)
    sinT = nc.declare_dram_parameter("sinT", [96, S], BF16, isOutput=False)
    tri = nc.declare_dram_parameter("tri", [KC, KC], BF16, isOutput=False)
    # block-diagonal 4x replicated identity: eye4[32b+k, m] = (m % 32 == k)
    eye4 = nc.declare_dram_parameter("eye4", [128, 128], BF16, isOutput=False)
    # per-head unnormalized projection partials + softmax denominators;
    # the division happens on the host (saves reciprocal+broadcast on-chip)
    out_part = nc.declare_dram_parameter("out_part", [2 * H, S], F32, isOutput=True)
    dens = nc.declare_dram_parameter("dens", [2, S], F32, isOutput=True)

    with tile.TileContext(nc) as tc:
        with (
            tc.tile_pool(name="const", bufs=1) as const,
            tc.tile_pool(name="qtp", bufs=4) as qtp,
            tc.tile_pool(name="qkp", bufs=3) as qkp,
            tc.tile_pool(name="ptp", bufs=5) as ptp,
            tc.tile_pool(name="smallp", bufs=6) as smallp,
            tc.tile_pool(name="ntp", bufs=4) as ntp,
            tc.tile_pool(name="outp", bufs=4) as outp,
            tc.tile_pool(name="ps_sc", bufs=3, space="PSUM") as ps_sc,
            tc.tile_pool(name="ps_pv", bufs=1, space="PSUM") as ps_pv,
            tc.tile_pool(name="ps_mm", bufs=1, space="PSUM") as ps_mm,
        ):
            # ---- persistent tiles + prologue DMAs
            hsT_sb = const.tile([128, 2, S], BF16)
            kT_rep = const.tile([128, S], BF16)
            v_all = const.tile([128, S // KC, HD + 1], BF16)
            cos_sb = const.tile([96, S], BF16)
            sin_sb = const.tile([96, S], BF16)
            tri_sb = const.tile([KC, KC], BF16)
            wqkT_sb = const.tile([128, 2, 96], BF16)
            wqkrotT_sb = const.tile([128, 2, 96], BF16)
            wvT_sb = const.tile([128, 2, HD], BF16)
            # head h's attention accumulator lives at partitions 64h..64h+33
            # (one PSUM bank for both heads); gt1 therefore sits at
            # partitions 64..97 so the lane-locked DVE/PE paths line up
            gt0_sb = const.tile([HD + 1, 2, 128], BF16)
            gt1_sb = const.tile([64 + HD + 1, 2, 128], BF16)
            eye4_sb = const.tile([128, 128], BF16)
            prime_b = const.tile([2, 16], BF16)

            # DMAs serialize on the sync engine (~600ns each) -- order the
            # prologue so block 0's dependencies land first, and batch wide.
            for c in range(2):
                nc.sync.dma_start(out=wqkT_sb[:, c, :], in_=wqkT[128 * c:128 * (c + 1), :])
                nc.sync.dma_start(out=wqkrotT_sb[:, c, :], in_=wqkrotT[128 * c:128 * (c + 1), :])
            nc.sync.dma_start(out=cos_sb[:, 0:QB], in_=cosT[:, 0:QB])
            nc.sync.dma_start(out=sin_sb[:, 0:QB], in_=sinT[:, 0:QB])
            nc.sync.dma_start(out=eye4_sb[:], in_=eye4[:])
            for c in range(2):
                nc.sync.dma_start(out=wvT_sb[:, c, :], in_=wvT[128 * c:128 * (c + 1), :])
            nc.sync.dma_start(out=tri_sb[:], in_=tri[:])
            # ones column of v_aug
            nc.vector.memset(v_all[:, :, 0:1], 1.0)
            # prime the gpsimd path (first call pays IRAM kernel load)
            nc.vector.memset(prime_b[:], 1.0)
            nc.gpsimd.tensor_mul(prime_b[:], prime_b[:], prime_b[:])

            def late_prologue():
                nc.sync.dma_start(out=cos_sb[:, QB:], in_=cosT[:, QB:])
                nc.sync.dma_start(out=sin_sb[:, QB:], in_=sinT[:, QB:])
                for c in range(2):
                    nc.sync.dma_start(out=gt0_sb[:, c, :],
                                      in_=gt0[:, 128 * c:128 * (c + 1)])
                    nc.sync.dma_start(out=gt1_sb[64:, c, :],
                                      in_=gt1[:, 128 * c:128 * (c + 1)])

            def prep_block(qb):
                """hidden-state load, q/k/v projections, RoPE, replication."""
                q0 = QB * qb
                sl = slice(q0, q0 + QB)
                nc.sync.dma_start(
                    out=hsT_sb[:, :, sl],
                    in_=hsT[:, sl].rearrange("(c p) q -> p c q", c=2))
                # q/k projections (plain + rotated) and RoPE; ps_mm has a
                # single bank, so each PSUM tile's reader precedes the next
                # allocation
                qkT = qkp.tile([96, QB], BF16, tag="qkT", name="qkT")
                rtmp = qkp.tile([96, QB], BF16, tag="rtmp", name="rtmp")
                p_qk = ps_mm.tile([96, QB], F32, tag="mm", name="p_qk")
                for c in range(2):
                    nc.tensor.matmul(p_qk[:], wqkT_sb[:, c, :], hsT_sb[:, c, sl],
                                     start=(c == 0), stop=(c == 1))
                nc.vector.tensor_mul(qkT[:], p_qk[:], cos_sb[:, sl])
                p_qkr = ps_mm.tile([96, QB], F32, tag="mm", name="p_qkr")
                for c in range(2):
                    nc.tensor.matmul(p_qkr[:], wqkrotT_sb[:, c, :], hsT_sb[:, c, sl],
                                     start=(c == 0), stop=(c == 1))
                nc.vector.tensor_mul(rtmp[:], p_qkr[:], sin_sb[:, sl])
                nc.vector.tensor_add(qkT[:], qkT[:], rtmp[:])

                # replicate qT (per head) and kT across the 4 row bands
                # (DMAs: keeps PSUM traffic off the DVE -- extra DVE PSUM
                # reads were measured to slow the scalar engine's exp ~20%)
                qt_h = [qtp.tile([128, QB], BF16, tag=f"qt{h}", name=f"qt{h}")
                        for h in range(2)]
                for t in range(4):
                    dst = slice(32 * t, 32 * (t + 1))
                    nc.sync.dma_start(out=qt_h[0][dst, :], in_=qkT[0:32, :])
                    nc.sync.dma_start(out=qt_h[1][dst, :], in_=qkT[32:64, :])
                    nc.sync.dma_start(out=kT_rep[dst, sl], in_=qkT[64:96, :])

                # v projection for this block's 4 s-chunks
                for s4 in range(4):
                    sblk = 4 * qb + s4
                    pv_ps = ps_mm.tile([128, HD], F32, tag="mm", name="vproj")
                    for c in range(2):
                        nc.tensor.matmul(
                            pv_ps[:], hsT_sb[:, c, KC * sblk:KC * (sblk + 1)],
                            wvT_sb[:, c, :], start=(c == 0), stop=(c == 1))
                    nc.vector.tensor_copy(v_all[:, sblk, 1:], pv_ps[:])
                return qt_h

            def make_epilogue(qb, pvacc):
                """Denominator + unnormalized per-head projection export,
                split into small parts injected one-per-iteration into the
                next block's stream so this PE/DVE work never starves the
                scalar engine. (host divides: G@(pv/den) == (G@pv)/den)"""
                q0 = QB * qb
                sl = slice(q0, q0 + QB)
                gts = [gt0_sb, gt1_sb]
                state = {}

                def part0():
                    # pvacc reads: must precede the next block's first PV
                    den_sb = smallp.tile([65, QB], F32, tag="den",
                                         name="den_sb")
                    nT = ntp.tile([64 + HD + 1, QB], BF16, tag="nT", name="nT")
                    for h in range(2):
                        r = 64 * h
                        nc.vector.tensor_copy(den_sb[r:r + 1, :],
                                              pvacc[r:r + 1, :])
                        nc.vector.tensor_copy(nT[r:r + HD + 1, :],
                                              pvacc[r:r + HD + 1, :])
                        nc.sync.dma_start(out=dens[h:h + 1, sl],
                                          in_=den_sb[r:r + 1, :])
                    state["nT"] = nT
                    state["po_sb"] = outp.tile([128, 4, QB], F32,
                                               name="po_sb")

                def make_po(h, mchunk):
                    def run():
                        nT, po_sb = state["nT"], state["po_sb"]
                        r = 64 * h
                        po = ps_mm.tile([128, QB], F32, tag="mm",
                                        name="outproj")
                        nc.tensor.matmul(po[:], gts[h][r:r + HD + 1, mchunk, :],
                                         nT[r:r + HD + 1, :],
                                         start=True, stop=True,
                                         tile_position=(r, 0))
                        nc.vector.tensor_copy(po_sb[:, 2 * h + mchunk, :],
                                              po[:])
                        if (h, mchunk) == (1, 1):
                            nc.sync.dma_start(
                                out=out_part[:, sl].rearrange(
                                    "(j p) q -> p j q", j=4),
                                in_=po_sb[:])
                    return run

                return [part0, make_po(0, 0), make_po(0, 1),
                        make_po(1, 0), make_po(1, 1)]

            qt_hold = {"qt": prep_block(0)}
            late_prologue()
            injections = []          # deferred finalize of the previous block
            iter_no = 0              # global iteration counter (strip rotation)

            def prep_into(q):
                qt_hold["qt"] = prep_block(q)

            for qb in range(NQB):
                qt_h = qt_hold["qt"]
                pvacc = ps_pv.tile([64 + HD + 1, QB], F32, tag="pvacc",
                                   name="pvacc")
                last_g = 2 * qb + 1

                def issue_pv(g, h, pt):
                    diag = (g >= 2 * qb)
                    for d in range(2):
                        ch = 2 * g + d
                        dg = ch - 4 * qb
                        cols = KC * dg if diag else 0
                        nc.tensor.matmul(
                            pvacc[64 * h:64 * h + HD + 1, cols:],
                            v_all[:, ch, :],
                            pt[:, QB * d + cols:QB * (d + 1)],
                            start=(g == 0 and d == 0),
                            stop=(g == last_g and d == 1),
                            skip_group_check=True)

                # PV issued two iterations behind exp: the PE never sits
                # behind a not-yet-exp'd tile in its in-order queue, so the
                # scalar engine stays saturated.
                pending = []
                idx = 0
                if qb + 1 < NQB:
                    injections.append(lambda q=qb + 1: prep_into(q))
                for g in range(2 * (qb + 1)):
                    for h in range(2):
                        # one deferred task per iteration, starting at the
                        # 3rd (epilogue part0 must precede this block's
                        # first PV write into the single pvacc bank)
                        if idx >= 2 and injections:
                            injections.pop(0)()
                        sc = ps_sc.tile([128, 2 * QB], F32, tag="sc", name="sc")
                        for d in range(2):
                            ch = 2 * g + d
                            # kT/qT live in all 4 row bands; rotate the pair
                            # of PE strips per iteration so consecutive
                            # iterations' score matmuls overlap on the array
                            t = (2 * iter_no + d) % 4
                            nc.tensor.matmul(
                                sc[:, QB * d:QB * (d + 1)],
                                kT_rep[32 * t:32 * (t + 1), KC * ch:KC * (ch + 1)],
                                qt_h[h][32 * t:32 * (t + 1), :],
                                start=True, stop=True, tile_position=(32 * t, 0))
                        iter_no += 1
                        pt = ptp.tile([128, 2 * QB], BF16, tag="pt", name="pt")
                        nc.scalar.activation(out=pt[:], in_=sc[:],
                                             func=mybir.ActivationFunctionType.Exp,
                                             scale=float(SCALE))
                        if g >= 2 * qb:
                            for d in range(2):
                                dg = 2 * g + d - 4 * qb
                                w0 = QB * d + KC * dg
                                nc.gpsimd.tensor_mul(
                                    pt[:, w0:w0 + KC], pt[:, w0:w0 + KC], tri_sb[:])
                        if len(pending) >= 2:
                            issue_pv(*pending.pop(0))
                        pending.append((g, h, pt))
                        idx += 1
                while pending:
                    issue_pv(*pending.pop(0))
                while injections:       # qb==0 is too short for all slots
                    injections.pop(0)()
                injections = make_epilogue(qb, pvacc)
            while injections:
                injections.pop(0)()

    _install_waitfix(nc)
    return nc


_NC_CACHE = {}


def _get_nc():
    if "nc" not in _NC_CACHE:
        _NC_CACHE["nc"] = _build_module()
    return _NC_CACHE["nc"]


# ---------------------------------------------------------------- host side
def _rope_tables():
    inv = 1.0 / (ROPE_BASE ** (np.arange(0, HD, 2, dtype=np.float64) / HD))
    t = np.arange(S, dtype=np.float64)
    freqs = np.outer(t, inv)                     # [S, 16]
    emb = np.concatenate([freqs, freqs], axis=1)  # [S, 32]
    cosT = np.cos(emb).T.astype(np.float32)      # [32, S]
    sinT = np.sin(emb).T.astype(np.float32)
    return np.tile(cosT, (3, 1)), np.tile(sinT, (3, 1))   # [96, S]


def _rot_rows(w):
    # rows of (rotate_half o) projection: row d<16 -> -w[d+16]; d>=16 -> w[d-16]
    return np.concatenate([-w[16:32], w[0:16]], axis=0)


def kernel(hidden_states, Wq, Wk, Wv, Wo):
    hidden_states = np.asarray(hidden_states, dtype=np.float32)
    Wq = np.asarray(Wq, dtype=np.float32)
    Wk = np.asarray(Wk, dtype=np.float32)
    Wv = np.asarray(Wv, dtype=np.float32)
    Wo = np.asarray(Wo, dtype=np.float32)

    cosT, sinT = _rope_tables()
    cosT = cosT.astype(NP_BF16)
    sinT = sinT.astype(NP_BF16)
    tri = (np.arange(KC)[:, None] <= np.arange(KC)[None, :]).astype(NP_BF16)
    eye4 = np.tile(np.eye(32, dtype=np.float32), (4, 4)).astype(NP_BF16)

    hsT_b = [np.ascontiguousarray(hidden_states[b].T).astype(NP_BF16)
             for b in range(B)]

    in_maps = []
    for core in range(8):
        b, j = core // 4, core % 4
        Wq_h = Wq[64 * j:64 * j + 64]            # [64, 256]
        Wk_j = Wk[32 * j:32 * j + 32]            # [32, 256]
        Wqk = np.concatenate([Wq_h, Wk_j], axis=0)           # [96, 256]
        Wqkrot = np.concatenate(
            [_rot_rows(Wq_h[0:32]), _rot_rows(Wq_h[32:64]), _rot_rows(Wk_j)],
            axis=0)
        G = Wo[:, 64 * j:64 * j + 64]            # [256, 64]
        gt0 = np.zeros((HD + 1, H), np.float32)
        gt0[1:] = G[:, 0:32].T
        gt1 = np.zeros((HD + 1, H), np.float32)
        gt1[1:] = G[:, 32:64].T
        in_maps.append({
            "hsT": hsT_b[b],
            "wqkT": np.ascontiguousarray(Wqk.T).astype(NP_BF16),
            "wqkrotT": np.ascontiguousarray(Wqkrot.T).astype(NP_BF16),
            "wvT": np.ascontiguousarray(Wv[32 * j:32 * j + 32].T).astype(NP_BF16),
            "gt0": gt0.astype(NP_BF16),
            "gt1": gt1.astype(NP_BF16),
            "cosT": cosT,
            "sinT": sinT,
            "tri": tri,
            "eye4": eye4,
        })

    nc = _get_nc()
    res = run_bass_kernel_spmd(nc, in_maps, list(range(8)), trace=False)

    out = np.empty((B, S, H), np.float32)
    for b in range(B):
        acc = np.zeros((H, S), np.float32)
        for j in range(4):
            r = res.results[4 * b + j]
            po, den = r["out_part"], r["dens"]
            for h in range(2):
                acc += po[H * h:H * (h + 1)] / den[h][None, :]
        out[b] = acc.T
    return out
